# revision 5
# baseline (speedup 1.0000x reference)
"""3-layer GCN (PyG GCNConv x3, relu between) on 8 Trainium2 NeuronCores.

Math: out = A*(relu(A*(relu(A*(xW1)+b1)W2+b2))W3)+b3 with A = D^-1/2(A+I)D^-1/2.
The edge norm factorizes as dinv[src]*dinv[dst], so per layer we compute
htilde = dinv * (input @ W) (dense, PE), AllGather htilde across the 8 cores,
then aggregation is an unweighted gather+segment-sum of htilde rows followed
by a dinv post-scale (+bias, relu). Nodes are degree-sorted and dealt
round-robin across cores so every core owns 12544 dst slots (98 blocks of
128 lanes) with near-uniform per-block degree; per-(block, int16-quarter)
gather slot lists are k-major so dma_gather lands edge k of lane p at
SBUF[p, k] and a single strided tensor_reduce does the segment sum. Padding
slots point at a per-slab zero row.
"""
import sys, time
sys.path.insert(0, "/opt/trn_rl_repo")
import numpy as np

N = 100_000
DIMS = [512, 128, 64, 32]
NCORES = 8
P = 128
SLOTS = 12544          # 98 blocks * 128 per core
NBLK = SLOTS // P      # 98
SLAB = SLOTS + 1       # +1 zero row per core slab
HROWS = NCORES * SLAB  # 100360
NQ = 4                 # int16-addressable quarters of the gathered table
QROWS = HROWS // NQ    # 25090
ZIDX = SLOTS           # quarter-local zero row (slab 2q, row 12544)
MAX_CALL_IDX = 3072
GDIMS = [128, 64, 64]  # gather widths (L3 padded 32->64 for 256B stride)

_CACHE = {}


# --------------------------------------------------------------------------
# host-side graph preprocessing
# --------------------------------------------------------------------------
def _preprocess(edge_index):
    src = np.asarray(edge_index[0], np.int64)
    dst = np.asarray(edge_index[1], np.int64)
    deg = np.bincount(dst, minlength=N).astype(np.int64) + 1  # + self loop
    dinv = (1.0 / np.sqrt(deg)).astype(np.float32)

    rank = np.argsort(-deg, kind="stable")
    pos = np.empty(N, np.int64); pos[rank] = np.arange(N)
    core_of = pos % NCORES
    slot_of = pos // NCORES

    S = np.concatenate([src, np.arange(N)])
    D_ = np.concatenate([dst, np.arange(N)])
    ec, eslot = core_of[D_], slot_of[D_]
    eblk, elane = eslot // P, eslot % P
    hrow = core_of[S] * SLAB + slot_of[S]
    eq, eqidx = hrow // QROWS, hrow % QROWS

    key = ((ec * NQ + eq) * NBLK + eblk) * P + elane
    order = np.argsort(key, kind="stable")
    ks = key[order]
    newgrp = np.r_[True, ks[1:] != ks[:-1]]
    first = np.flatnonzero(newgrp)
    within = np.arange(len(ks)) - first[np.cumsum(newgrp) - 1]

    cnt = np.zeros(NCORES * NQ * NBLK * P, np.int64)
    np.add.at(cnt, key, 1)
    K = cnt.reshape(NCORES, NQ, NBLK, P).max(axis=(0, 3))  # [NQ, NBLK]
    K = np.maximum(K, 1)

    packs = []  # (q, j0, j1, col_offs, nidx)
    for q in range(NQ):
        j = 0
        while j < NBLK:
            tot, j1, offs = 0, j, []
            while j1 < NBLK and (tot + K[q, j1]) * P <= MAX_CALL_IDX:
                offs.append(tot); tot += int(K[q, j1]); j1 += 1
            if j1 == j:  # single oversized block
                offs, tot, j1 = [0], int(K[q, j]), j + 1
            packs.append((q, j, j1, offs, tot * P))
            j = j1
    packs.sort(key=lambda pk: (pk[1], pk[0]))

    blkq = [[None] * NQ for _ in range(NBLK)]
    cidm = np.full((NQ, NBLK), -1, np.int64)
    offm = np.zeros((NQ, NBLK), np.int64)
    cbase = np.zeros(len(packs), np.int64)
    acc = 0
    for cid, (q, j0, j1, offs, nidx) in enumerate(packs):
        cbase[cid] = acc; acc += nidx
        for t, j in enumerate(range(j0, j1)):
            blkq[j][q] = (cid, offs[t], int(K[q, j]))
            cidm[q, j], offm[q, j] = cid, offs[t]
    total_idx = acc

    req = np.full((NCORES, total_idx), ZIDX, np.int64)
    oc, oq, ob, ol = ec[order], eq[order], eblk[order], elane[order]
    ecall = cidm[oq, ob]
    ekoff = offm[oq, ob] + within
    req[oc, cbase[ecall] + ekoff * P + ol] = eqidx[order]
    assert req.max() < 32768

    wrapped = np.empty((NCORES, P, total_idx // 16), np.int16)
    for c in range(NCORES):
        col = 0
        for cid, pk in enumerate(packs):
            L = req[c, cbase[cid]:cbase[cid] + pk[4]]
            w = L.reshape(-1, 16).T.astype(np.int16)
            wrapped[c, :, col:col + pk[4] // 16] = np.tile(w, (8, 1))
            col += pk[4] // 16

    ids = np.full((NCORES, SLOTS), -1, np.int64)
    ids[core_of, slot_of] = np.arange(N)
    dinv_t = np.zeros((NCORES, P, NBLK), np.float32)
    for c in range(NCORES):
        v = ids[c]
        dv = np.where(v >= 0, dinv[np.maximum(v, 0)], 0.0).astype(np.float32)
        dinv_t[c] = dv.reshape(NBLK, P).T
    return dict(ids=ids, packs=packs, blkq=blkq, cbase=cbase,
                wrapped=wrapped, dinv_t=dinv_t, total_idx=total_idx)


# --------------------------------------------------------------------------
# bass program
# --------------------------------------------------------------------------
def _build(pre):
    from concourse import bass, bacc, mybir, tile
    from concourse.library_config import mlp
    from concourse.masks import make_identity
    AL = mybir.AluOpType
    f32, i16 = mybir.dt.float32, mybir.dt.int16
    packs, blkq, cbase = pre["packs"], pre["blkq"], pre["cbase"]
    total_idx = pre["total_idx"]

    nc = bacc.Bacc("TRN2", target_bir_lowering=False, debug=False,
                   num_devices=NCORES)
    xT_in = nc.dram_tensor("xT", (DIMS[0], SLOTS), f32, kind="ExternalInput")
    idx_in = nc.dram_tensor("gidx", (P, total_idx // 16), i16, kind="ExternalInput")
    dinv_in = nc.dram_tensor("dinv", (P, NBLK), f32, kind="ExternalInput")
    w_in = [nc.dram_tensor(f"W{i+1}", (DIMS[i], DIMS[i + 1]), f32, kind="ExternalInput") for i in range(3)]
    b_in = [nc.dram_tensor(f"b{i+1}", (P, DIMS[i + 1]), f32, kind="ExternalInput") for i in range(3)]
    out_t = nc.dram_tensor("out", (SLOTS, DIMS[3]), f32, kind="ExternalOutput")

    with tile.TileContext(nc) as tc:
        with tc.tile_pool(name="const", bufs=1) as const, \
             tc.tile_pool(name="gath", bufs=8) as gpool, \
             tc.tile_pool(name="work", bufs=3) as work, \
             tc.tile_pool(name="lhs", bufs=3) as lhs, \
             tc.tile_pool(name="pps", bufs=2, space="PSUM") as pps, \
             tc.tile_pool(name="ppt", bufs=2, space="PSUM") as ppt, \
             tc.tile_pool(name="dram", bufs=1, space="DRAM") as dram:

            nc.gpsimd.load_library(mlp)
            idx_t = const.tile([P, total_idx // 16], i16, tag="idx")
            nc.sync.dma_start(out=idx_t[:], in_=idx_in[:, :])
            dinv_t = const.tile([P, NBLK], f32, tag="dinv")
            nc.sync.dma_start(out=dinv_t[:], in_=dinv_in[:, :])
            ident = const.tile([P, P], f32, tag="ident")
            make_identity(nc, ident[:])
            zt = const.tile([P, P], f32, tag="zero")
            nc.vector.memset(zt[:], 0.0)
            w1t = [const.tile([P, DIMS[1]], f32, tag=f"w1_{k}", name=f"w1_{k}") for k in range(4)]
            for k in range(4):
                nc.sync.dma_start(out=w1t[k][:], in_=w_in[0][k * P:(k + 1) * P, :])
            w2t = const.tile([DIMS[1], DIMS[2]], f32, tag="w2")
            nc.sync.dma_start(out=w2t[:], in_=w_in[1][:, :])
            w3t = const.tile([DIMS[2], DIMS[3]], f32, tag="w3")
            nc.sync.dma_start(out=w3t[:], in_=w_in[2][:, :])
            bt = []
            for i in range(3):
                t = const.tile([P, DIMS[i + 1]], f32, tag=f"b{i}", name=f"bt{i}")
                nc.sync.dma_start(out=t[:], in_=b_in[i][:, :])
                bt.append(t)

            slabs = [dram.tile([SLAB, GDIMS[i]], f32, tag=f"slab{i}", name=f"slab{i}") for i in range(3)]
            hbufs = [dram.tile([HROWS, GDIMS[i]], f32, tag=f"hbuf{i}", name=f"hbuf{i}") for i in range(3)]

            def aggregate(layer):
                d = GDIMS[layer]
                nc.gpsimd.collective_compute(
                    "AllGather", AL.bypass,
                    replica_groups=[list(range(NCORES))],
                    ins=[slabs[layer].opt()], outs=[hbufs[layer].opt()])
                tiles = {}
                for cid, (q, j0, j1, offs, nidx) in enumerate(packs):
                    gt = gpool.tile([P, nidx // P, d], f32, tag="gt")
                    nc.gpsimd.dma_gather(
                        out_ap=gt[:],
                        in_ap=hbufs[layer][q * QROWS:(q + 1) * QROWS, :],
                        idxs_ap=idx_t[:, cbase[cid] // 16:(cbase[cid] + nidx) // 16],
                        num_idxs=nidx, num_idxs_reg=nidx, elem_size=d,
                        single_packet=False)
                    tiles[cid] = gt
                return tiles

            def reduce_block(tiles, j, dout):
                s = work.tile([P, dout], f32, tag="ssum")
                for q in range(NQ):
                    cid, coff, kq = blkq[j][q]
                    gt = tiles[cid]
                    view = gt[:, coff:coff + kq, :dout].rearrange("p k d -> p d k")
                    if q == 0:
                        nc.vector.tensor_reduce(out=s[:], in_=view,
                            axis=mybir.AxisListType.X, op=AL.add)
                    else:
                        tmp = work.tile([P, dout], f32, tag="rtmp")
                        nc.vector.tensor_reduce(out=tmp[:], in_=view,
                            axis=mybir.AxisListType.X, op=AL.add)
                        nc.vector.tensor_tensor(out=s[:], in0=s[:], in1=tmp[:], op=AL.add)
                return s

            def dscale(dst_ap, src_ap, j, d):
                nc.vector.tensor_tensor(
                    out=dst_ap, in0=src_ap,
                    in1=dinv_t[:, j:j + 1].to_broadcast([P, d]), op=AL.mult)

            # ---- L1 transform
            for j in range(NBLK):
                ps = pps.tile([P, DIMS[1]], f32, space="PSUM", tag="tps")
                for k in range(4):
                    lt = lhs.tile([P, P], f32, tag="xT")
                    nc.sync.dma_start(out=lt[:], in_=xT_in[k * P:(k + 1) * P, j * P:(j + 1) * P])
                    nc.tensor.matmul(out=ps[:], lhsT=lt[:], rhs=w1t[k][:],
                                     start=(k == 0), stop=(k == 3))
                ht = work.tile([P, DIMS[1]], f32, tag="hrow")
                dscale(ht[:], ps[:], j, DIMS[1])
                nc.sync.dma_start(out=slabs[0][j * P:(j + 1) * P, :], in_=ht[:])
            nc.sync.dma_start(out=slabs[0][SLOTS:SLOTS + 1, :], in_=zt[0:1, :GDIMS[0]])

            # ---- L1 aggregate + L2 transform
            tiles = aggregate(0)
            for j in range(NBLK):
                s = reduce_block(tiles, j, DIMS[1])
                dscale(s[:], s[:], j, DIMS[1])
                nc.vector.tensor_tensor(out=s[:], in0=s[:], in1=bt[0][:], op=AL.add)
                nc.vector.tensor_scalar_max(out=s[:], in0=s[:], scalar1=0.0)
                pt = ppt.tile([P, P], f32, space="PSUM", tag="trp")
                nc.tensor.transpose(out=pt[:DIMS[1], :], in_=s[:], identity=ident[:])
                sT = work.tile([DIMS[1], P], f32, tag="sT")
                nc.vector.tensor_copy(out=sT[:], in_=pt[:DIMS[1], :])
                ps = pps.tile([P, DIMS[2]], f32, space="PSUM", tag="tps2")
                nc.tensor.matmul(out=ps[:], lhsT=sT[:], rhs=w2t[:], start=True, stop=True)
                ht = work.tile([P, DIMS[2]], f32, tag="h2row")
                dscale(ht[:], ps[:], j, DIMS[2])
                nc.sync.dma_start(out=slabs[1][j * P:(j + 1) * P, :], in_=ht[:])
            nc.sync.dma_start(out=slabs[1][SLOTS:SLOTS + 1, :], in_=zt[0:1, :GDIMS[1]])

            # ---- L2 aggregate + L3 transform
            tiles = aggregate(1)
            for j in range(NBLK):
                s = reduce_block(tiles, j, DIMS[2])
                dscale(s[:], s[:], j, DIMS[2])
                nc.vector.tensor_tensor(out=s[:], in0=s[:], in1=bt[1][:], op=AL.add)
                nc.vector.tensor_scalar_max(out=s[:], in0=s[:], scalar1=0.0)
                pt = ppt.tile([P, P], f32, space="PSUM", tag="trp")
                nc.tensor.transpose(out=pt[:DIMS[2], :], in_=s[:], identity=ident[:])
                sT = work.tile([DIMS[2], P], f32, tag="s3T")
                nc.vector.tensor_copy(out=sT[:], in_=pt[:DIMS[2], :])
                ps = pps.tile([P, DIMS[3]], f32, space="PSUM", tag="tps3")
                nc.tensor.matmul(out=ps[:], lhsT=sT[:], rhs=w3t[:], start=True, stop=True)
                ht = work.tile([P, DIMS[3]], f32, tag="h3row")
                dscale(ht[:], ps[:], j, DIMS[3])
                nc.sync.dma_start(out=slabs[2][j * P:(j + 1) * P, :DIMS[3]], in_=ht[:])
                nc.sync.dma_start(out=slabs[2][j * P:(j + 1) * P, DIMS[3]:], in_=zt[:, :GDIMS[2] - DIMS[3]])
            nc.sync.dma_start(out=slabs[2][SLOTS:SLOTS + 1, :], in_=zt[0:1, :GDIMS[2]])

            # ---- L3 aggregate -> output
            tiles = aggregate(2)
            for j in range(NBLK):
                s = reduce_block(tiles, j, DIMS[3])
                dscale(s[:], s[:], j, DIMS[3])
                nc.vector.tensor_tensor(out=s[:], in0=s[:], in1=bt[2][:], op=AL.add)
                nc.sync.dma_start(out=out_t[j * P:(j + 1) * P, :], in_=s[:])
    nc.compile()
    return nc


# --------------------------------------------------------------------------
# SPMD runner (shard_map over 8 axon cores, reusable jitted executable)
# --------------------------------------------------------------------------
class _Runner:
    def __init__(self, nc, n_cores=NCORES):
        import jax
        from jax.sharding import Mesh, PartitionSpec
        from jax.experimental.shard_map import shard_map
        from concourse import bass2jax, mybir
        bass2jax.install_neuronx_cc_hook()
        self.jax = jax
        self.n_cores = n_cores
        pname = nc.partition_id_tensor.name if nc.partition_id_tensor else None
        in_names, out_names, out_avals, zero_outs = [], [], [], []
        for alloc in nc.m.functions[0].allocations:
            if not isinstance(alloc, mybir.MemoryLocationSet):
                continue
            name = alloc.memorylocations[0].name
            if alloc.kind == "ExternalInput":
                if name != pname:
                    in_names.append(name)
            elif alloc.kind == "ExternalOutput":
                out_names.append(name)
                out_avals.append(jax.core.ShapedArray(tuple(alloc.tensor_shape), mybir.dt.np(alloc.dtype)))
                zero_outs.append(np.zeros(tuple(alloc.tensor_shape), mybir.dt.np(alloc.dtype)))
        self.in_names, self.out_names = in_names, out_names
        self.out_avals, self.zero_outs = out_avals, zero_outs
        n_params, n_outs = len(in_names), len(out_names)
        all_in = in_names + out_names + ([pname] if pname else [])

        def _body(*args):
            operands = list(args)
            if pname:
                operands.append(bass2jax.partition_id_tensor())
            outs = bass2jax._bass_exec_p.bind(
                *operands, out_avals=tuple(out_avals), in_names=tuple(all_in),
                out_names=tuple(out_names), lowering_input_output_aliases=(),
                sim_require_finite=True, sim_require_nnan=True, nc=nc)
            return tuple(outs)

        devices = jax.devices()[:n_cores]
        self.mesh = Mesh(np.asarray(devices), ("core",))
        self.pspec = PartitionSpec("core")
        self.fn = jax.jit(
            shard_map(_body, mesh=self.mesh,
                      in_specs=(self.pspec,) * (n_params + n_outs),
                      out_specs=(self.pspec,) * n_outs, check_rep=False),
            donate_argnums=tuple(range(n_params, n_params + n_outs)),
            keep_unused=True)

    def place(self, in_maps):
        sh = self.jax.sharding.NamedSharding(self.mesh, self.pspec)
        return [self.jax.device_put(
                    np.concatenate([np.asarray(in_maps[c][n]) for c in range(self.n_cores)], axis=0), sh)
                for n in self.in_names]

    def run(self, args):
        sh = self.jax.sharding.NamedSharding(self.mesh, self.pspec)
        zeros = [self.jax.device_put(
                    np.zeros((self.n_cores * z.shape[0], *z.shape[1:]), z.dtype), sh)
                 for z in self.zero_outs]
        outs = self.fn(*args, *zeros)
        self.jax.block_until_ready(outs)
        return outs

    def results(self, outs):
        return [{n: np.asarray(outs[i]).reshape(self.n_cores, *self.out_avals[i].shape)[c]
                 for i, n in enumerate(self.out_names)}
                for c in range(self.n_cores)]


# --------------------------------------------------------------------------
def _get(edge_index):
    key = hash(np.asarray(edge_index)[:, ::997].tobytes())
    if key not in _CACHE:
        pre = _preprocess(edge_index)
        nc = _build(pre)
        _CACHE[key] = (pre, _Runner(nc))
    return _CACHE[key]


def kernel(x, edge_index, W1, b1, W2, b2, W3, b3):
    pre, runner = _get(edge_index)
    x = np.asarray(x, np.float32)
    ids = pre["ids"]
    in_maps = []
    for c in range(NCORES):
        v = ids[c]
        xc = np.zeros((SLOTS, DIMS[0]), np.float32)
        m = v >= 0
        xc[m] = x[v[m]]
        in_maps.append({
            "xT": np.ascontiguousarray(xc.T),
            "gidx": pre["wrapped"][c],
            "dinv": pre["dinv_t"][c],
            "W1": np.asarray(W1, np.float32), "W2": np.asarray(W2, np.float32),
            "W3": np.asarray(W3, np.float32),
            "b1": np.tile(np.asarray(b1, np.float32)[None, :], (P, 1)),
            "b2": np.tile(np.asarray(b2, np.float32)[None, :], (P, 1)),
            "b3": np.tile(np.asarray(b3, np.float32)[None, :], (P, 1)),
        })
    args = runner.place(in_maps)
    outs = runner.run(args)
    res = runner.results(outs)
    full = np.zeros((N, DIMS[3]), np.float32)
    for c in range(NCORES):
        v = ids[c]
        m = v >= 0
        full[v[m]] = res[c]["out"][m]
    return full


# revision 6
# speedup vs baseline: 1.1649x; 1.1649x over previous
"""3-layer GCN (PyG GCNConv x3, relu between) on 8 Trainium2 NeuronCores.

Math: out = A*(relu(A*(relu(A*(xW1)+b1)W2+b2))W3)+b3 with A = D^-1/2(A+I)D^-1/2.
The edge norm factorizes as dinv[src]*dinv[dst], so per layer we compute
htilde = dinv * (input @ W) (dense, PE), AllGather htilde across the 8 cores,
then aggregation is an unweighted gather+segment-sum of htilde rows followed
by a dinv post-scale (+bias, relu). Nodes are degree-sorted and dealt
round-robin across cores so every core owns 12544 dst slots (98 blocks of
128 lanes) with near-uniform per-block degree; per-(block, int16-quarter)
gather slot lists are k-major so dma_gather lands edge k of lane p at
SBUF[p, k] and a single strided tensor_reduce does the segment sum. Padding
slots point at a per-slab zero row.
"""
import sys, time
sys.path.insert(0, "/opt/trn_rl_repo")
import numpy as np

N = 100_000
DIMS = [512, 128, 64, 32]
NCORES = 8
P = 128
SLOTS = 12544          # 98 blocks * 128 per core
NBLK = SLOTS // P      # 98
SLAB = SLOTS + 1       # +1 zero row per core slab
HROWS = NCORES * SLAB  # 100360
NQ = 4                 # int16-addressable quarters of the gathered table
QROWS = HROWS // NQ    # 25090
ZIDX = SLOTS           # quarter-local zero row (slab 2q, row 12544)
MAX_CALL_IDX = 3072
GDIMS = [128, 64, 64]  # gather widths (L3 padded 32->64 for 256B stride)

_CACHE = {}


# --------------------------------------------------------------------------
# host-side graph preprocessing
# --------------------------------------------------------------------------
def _preprocess(edge_index):
    src = np.asarray(edge_index[0], np.int64)
    dst = np.asarray(edge_index[1], np.int64)
    deg = np.bincount(dst, minlength=N).astype(np.int64) + 1  # + self loop
    dinv = (1.0 / np.sqrt(deg)).astype(np.float32)

    rank = np.argsort(-deg, kind="stable")
    pos = np.empty(N, np.int64); pos[rank] = np.arange(N)
    core_of = pos % NCORES
    slot_of = pos // NCORES

    S = np.concatenate([src, np.arange(N)])
    D_ = np.concatenate([dst, np.arange(N)])
    ec, eslot = core_of[D_], slot_of[D_]
    eblk, elane = eslot // P, eslot % P
    hrow = core_of[S] * SLAB + slot_of[S]
    eq, eqidx = hrow // QROWS, hrow % QROWS

    key = ((ec * NQ + eq) * NBLK + eblk) * P + elane
    order = np.argsort(key, kind="stable")
    ks = key[order]
    newgrp = np.r_[True, ks[1:] != ks[:-1]]
    first = np.flatnonzero(newgrp)
    within = np.arange(len(ks)) - first[np.cumsum(newgrp) - 1]

    cnt = np.zeros(NCORES * NQ * NBLK * P, np.int64)
    np.add.at(cnt, key, 1)
    K = cnt.reshape(NCORES, NQ, NBLK, P).max(axis=(0, 3))  # [NQ, NBLK]
    K = np.maximum(K, 1)

    packs = []  # (q, j0, j1, col_offs, nidx)
    for q in range(NQ):
        j = 0
        while j < NBLK:
            tot, j1, offs = 0, j, []
            while j1 < NBLK and (tot + K[q, j1]) * P <= MAX_CALL_IDX:
                offs.append(tot); tot += int(K[q, j1]); j1 += 1
            if j1 == j:  # single oversized block
                offs, tot, j1 = [0], int(K[q, j]), j + 1
            packs.append((q, j, j1, offs, tot * P))
            j = j1
    packs.sort(key=lambda pk: (pk[1], pk[0]))

    blkq = [[None] * NQ for _ in range(NBLK)]
    cidm = np.full((NQ, NBLK), -1, np.int64)
    offm = np.zeros((NQ, NBLK), np.int64)
    cbase = np.zeros(len(packs), np.int64)
    acc = 0
    for cid, (q, j0, j1, offs, nidx) in enumerate(packs):
        cbase[cid] = acc; acc += nidx
        for t, j in enumerate(range(j0, j1)):
            blkq[j][q] = (cid, offs[t], int(K[q, j]))
            cidm[q, j], offm[q, j] = cid, offs[t]
    total_idx = acc

    req = np.full((NCORES, total_idx), ZIDX, np.int64)
    oc, oq, ob, ol = ec[order], eq[order], eblk[order], elane[order]
    ecall = cidm[oq, ob]
    ekoff = offm[oq, ob] + within
    req[oc, cbase[ecall] + ekoff * P + ol] = eqidx[order]
    assert req.max() < 32768

    wrapped = np.empty((NCORES, P, total_idx // 16), np.int16)
    for c in range(NCORES):
        col = 0
        for cid, pk in enumerate(packs):
            L = req[c, cbase[cid]:cbase[cid] + pk[4]]
            w = L.reshape(-1, 16).T.astype(np.int16)
            wrapped[c, :, col:col + pk[4] // 16] = np.tile(w, (8, 1))
            col += pk[4] // 16

    ids = np.full((NCORES, SLOTS), -1, np.int64)
    ids[core_of, slot_of] = np.arange(N)
    dinv_t = np.zeros((NCORES, P, NBLK), np.float32)
    for c in range(NCORES):
        v = ids[c]
        dv = np.where(v >= 0, dinv[np.maximum(v, 0)], 0.0).astype(np.float32)
        dinv_t[c] = dv.reshape(NBLK, P).T
    return dict(ids=ids, packs=packs, blkq=blkq, cbase=cbase,
                wrapped=wrapped, dinv_t=dinv_t, total_idx=total_idx)


# --------------------------------------------------------------------------
# bass program
# --------------------------------------------------------------------------
def _build(pre):
    from concourse import bass, bacc, mybir, tile
    from concourse.library_config import mlp
    from concourse.masks import make_identity
    AL = mybir.AluOpType
    f32, i16 = mybir.dt.float32, mybir.dt.int16
    packs, blkq, cbase = pre["packs"], pre["blkq"], pre["cbase"]
    total_idx = pre["total_idx"]

    nc = bacc.Bacc("TRN2", target_bir_lowering=False, debug=False,
                   num_devices=NCORES)
    xT_in = nc.dram_tensor("xT", (DIMS[0], SLOTS), f32, kind="ExternalInput")
    idx_in = nc.dram_tensor("gidx", (P, total_idx // 16), i16, kind="ExternalInput")
    dinv_in = nc.dram_tensor("dinv", (P, NBLK), f32, kind="ExternalInput")
    w_in = [nc.dram_tensor(f"W{i+1}", (DIMS[i], DIMS[i + 1]), f32, kind="ExternalInput") for i in range(3)]
    b_in = [nc.dram_tensor(f"b{i+1}", (P, DIMS[i + 1]), f32, kind="ExternalInput") for i in range(3)]
    out_t = nc.dram_tensor("out", (SLOTS, DIMS[3]), f32, kind="ExternalOutput")

    with tile.TileContext(nc) as tc:
        with tc.tile_pool(name="const", bufs=1) as const, \
             tc.tile_pool(name="gath", bufs=8) as gpool, \
             tc.tile_pool(name="work", bufs=3) as work, \
             tc.tile_pool(name="lhs", bufs=3) as lhs, \
             tc.tile_pool(name="pps", bufs=2, space="PSUM") as pps, \
             tc.tile_pool(name="ppt", bufs=2, space="PSUM") as ppt, \
             tc.tile_pool(name="dram", bufs=1, space="DRAM") as dram:

            nc.gpsimd.load_library(mlp)
            idx_t = const.tile([P, total_idx // 16], i16, tag="idx")
            nc.sync.dma_start(out=idx_t[:], in_=idx_in[:, :])
            dinv_t = const.tile([P, NBLK], f32, tag="dinv")
            nc.sync.dma_start(out=dinv_t[:], in_=dinv_in[:, :])
            ident = const.tile([P, P], f32, tag="ident")
            make_identity(nc, ident[:])
            zt = const.tile([P, P], f32, tag="zero")
            nc.vector.memset(zt[:], 0.0)
            w1t = [const.tile([P, DIMS[1]], f32, tag=f"w1_{k}", name=f"w1_{k}") for k in range(4)]
            for k in range(4):
                nc.sync.dma_start(out=w1t[k][:], in_=w_in[0][k * P:(k + 1) * P, :])
            w2t = const.tile([DIMS[1], DIMS[2]], f32, tag="w2")
            nc.sync.dma_start(out=w2t[:], in_=w_in[1][:, :])
            w3t = const.tile([DIMS[2], DIMS[3]], f32, tag="w3")
            nc.sync.dma_start(out=w3t[:], in_=w_in[2][:, :])
            bt = []
            for i in range(3):
                t = const.tile([P, DIMS[i + 1]], f32, tag=f"b{i}", name=f"bt{i}")
                nc.sync.dma_start(out=t[:], in_=b_in[i][:, :])
                bt.append(t)

            slabs = [dram.tile([SLAB, GDIMS[i]], f32, tag=f"slab{i}", name=f"slab{i}") for i in range(3)]
            hbufs = [dram.tile([HROWS, GDIMS[i]], f32, tag=f"hbuf{i}", name=f"hbuf{i}", addr_space="Shared") for i in range(3)]

            def aggregate(layer):
                d = GDIMS[layer]
                nc.gpsimd.collective_compute(
                    "AllGather", AL.bypass,
                    replica_groups=[list(range(NCORES))],
                    ins=[slabs[layer].opt()], outs=[hbufs[layer].opt()])
                tiles = {}
                for cid, (q, j0, j1, offs, nidx) in enumerate(packs):
                    gt = gpool.tile([P, nidx // P, d], f32, tag="gt")
                    nc.gpsimd.dma_gather(
                        out_ap=gt[:],
                        in_ap=hbufs[layer][q * QROWS:(q + 1) * QROWS, :],
                        idxs_ap=idx_t[:, cbase[cid] // 16:(cbase[cid] + nidx) // 16],
                        num_idxs=nidx, num_idxs_reg=nidx, elem_size=d,
                        single_packet=False)
                    tiles[cid] = gt
                return tiles

            def reduce_block(tiles, j, dout):
                s = work.tile([P, dout], f32, tag="ssum")
                for q in range(NQ):
                    cid, coff, kq = blkq[j][q]
                    gt = tiles[cid]
                    view = gt[:, coff:coff + kq, :dout].rearrange("p k d -> p d k")
                    if q == 0:
                        nc.vector.tensor_reduce(out=s[:], in_=view,
                            axis=mybir.AxisListType.X, op=AL.add)
                    else:
                        tmp = work.tile([P, dout], f32, tag="rtmp")
                        nc.vector.tensor_reduce(out=tmp[:], in_=view,
                            axis=mybir.AxisListType.X, op=AL.add)
                        nc.vector.tensor_tensor(out=s[:], in0=s[:], in1=tmp[:], op=AL.add)
                return s

            def dscale(dst_ap, src_ap, j, d):
                nc.vector.tensor_tensor(
                    out=dst_ap, in0=src_ap,
                    in1=dinv_t[:, j:j + 1].to_broadcast([P, d]), op=AL.mult)

            # ---- L1 transform
            for j in range(NBLK):
                ps = pps.tile([P, DIMS[1]], f32, space="PSUM", tag="tps")
                for k in range(4):
                    lt = lhs.tile([P, P], f32, tag="xT")
                    nc.sync.dma_start(out=lt[:], in_=xT_in[k * P:(k + 1) * P, j * P:(j + 1) * P])
                    nc.tensor.matmul(out=ps[:], lhsT=lt[:], rhs=w1t[k][:],
                                     start=(k == 0), stop=(k == 3))
                ht = work.tile([P, DIMS[1]], f32, tag="hrow")
                dscale(ht[:], ps[:], j, DIMS[1])
                nc.sync.dma_start(out=slabs[0][j * P:(j + 1) * P, :], in_=ht[:])
            nc.sync.dma_start(out=slabs[0][SLOTS:SLOTS + 1, :], in_=zt[0:1, :GDIMS[0]])

            # ---- L1 aggregate + L2 transform
            tiles = aggregate(0)
            for j in range(NBLK):
                s = reduce_block(tiles, j, DIMS[1])
                dscale(s[:], s[:], j, DIMS[1])
                nc.vector.tensor_tensor(out=s[:], in0=s[:], in1=bt[0][:], op=AL.add)
                nc.vector.tensor_scalar_max(out=s[:], in0=s[:], scalar1=0.0)
                pt = ppt.tile([P, P], f32, space="PSUM", tag="trp")
                nc.tensor.transpose(out=pt[:DIMS[1], :], in_=s[:], identity=ident[:])
                sT = work.tile([DIMS[1], P], f32, tag="sT")
                nc.vector.tensor_copy(out=sT[:], in_=pt[:DIMS[1], :])
                ps = pps.tile([P, DIMS[2]], f32, space="PSUM", tag="tps2")
                nc.tensor.matmul(out=ps[:], lhsT=sT[:], rhs=w2t[:], start=True, stop=True)
                ht = work.tile([P, DIMS[2]], f32, tag="h2row")
                dscale(ht[:], ps[:], j, DIMS[2])
                nc.sync.dma_start(out=slabs[1][j * P:(j + 1) * P, :], in_=ht[:])
            nc.sync.dma_start(out=slabs[1][SLOTS:SLOTS + 1, :], in_=zt[0:1, :GDIMS[1]])

            # ---- L2 aggregate + L3 transform
            tiles = aggregate(1)
            for j in range(NBLK):
                s = reduce_block(tiles, j, DIMS[2])
                dscale(s[:], s[:], j, DIMS[2])
                nc.vector.tensor_tensor(out=s[:], in0=s[:], in1=bt[1][:], op=AL.add)
                nc.vector.tensor_scalar_max(out=s[:], in0=s[:], scalar1=0.0)
                pt = ppt.tile([P, P], f32, space="PSUM", tag="trp")
                nc.tensor.transpose(out=pt[:DIMS[2], :], in_=s[:], identity=ident[:])
                sT = work.tile([DIMS[2], P], f32, tag="s3T")
                nc.vector.tensor_copy(out=sT[:], in_=pt[:DIMS[2], :])
                ps = pps.tile([P, DIMS[3]], f32, space="PSUM", tag="tps3")
                nc.tensor.matmul(out=ps[:], lhsT=sT[:], rhs=w3t[:], start=True, stop=True)
                ht = work.tile([P, DIMS[3]], f32, tag="h3row")
                dscale(ht[:], ps[:], j, DIMS[3])
                nc.sync.dma_start(out=slabs[2][j * P:(j + 1) * P, :DIMS[3]], in_=ht[:])
                nc.sync.dma_start(out=slabs[2][j * P:(j + 1) * P, DIMS[3]:], in_=zt[:, :GDIMS[2] - DIMS[3]])
            nc.sync.dma_start(out=slabs[2][SLOTS:SLOTS + 1, :], in_=zt[0:1, :GDIMS[2]])

            # ---- L3 aggregate -> output
            tiles = aggregate(2)
            for j in range(NBLK):
                s = reduce_block(tiles, j, DIMS[3])
                dscale(s[:], s[:], j, DIMS[3])
                nc.vector.tensor_tensor(out=s[:], in0=s[:], in1=bt[2][:], op=AL.add)
                nc.sync.dma_start(out=out_t[j * P:(j + 1) * P, :], in_=s[:])
    nc.compile()
    return nc


# --------------------------------------------------------------------------
# SPMD runner (shard_map over 8 axon cores, reusable jitted executable)
# --------------------------------------------------------------------------
class _Runner:
    def __init__(self, nc, n_cores=NCORES):
        import jax
        from jax.sharding import Mesh, PartitionSpec
        from jax.experimental.shard_map import shard_map
        from concourse import bass2jax, mybir
        bass2jax.install_neuronx_cc_hook()
        self.jax = jax
        self.n_cores = n_cores
        pname = nc.partition_id_tensor.name if nc.partition_id_tensor else None
        in_names, out_names, out_avals, zero_outs = [], [], [], []
        for alloc in nc.m.functions[0].allocations:
            if not isinstance(alloc, mybir.MemoryLocationSet):
                continue
            name = alloc.memorylocations[0].name
            if alloc.kind == "ExternalInput":
                if name != pname:
                    in_names.append(name)
            elif alloc.kind == "ExternalOutput":
                out_names.append(name)
                out_avals.append(jax.core.ShapedArray(tuple(alloc.tensor_shape), mybir.dt.np(alloc.dtype)))
                zero_outs.append(np.zeros(tuple(alloc.tensor_shape), mybir.dt.np(alloc.dtype)))
        self.in_names, self.out_names = in_names, out_names
        self.out_avals, self.zero_outs = out_avals, zero_outs
        n_params, n_outs = len(in_names), len(out_names)
        all_in = in_names + out_names + ([pname] if pname else [])

        def _body(*args):
            operands = list(args)
            if pname:
                operands.append(bass2jax.partition_id_tensor())
            outs = bass2jax._bass_exec_p.bind(
                *operands, out_avals=tuple(out_avals), in_names=tuple(all_in),
                out_names=tuple(out_names), lowering_input_output_aliases=(),
                sim_require_finite=True, sim_require_nnan=True, nc=nc)
            return tuple(outs)

        devices = jax.devices()[:n_cores]
        self.mesh = Mesh(np.asarray(devices), ("core",))
        self.pspec = PartitionSpec("core")
        self.fn = jax.jit(
            shard_map(_body, mesh=self.mesh,
                      in_specs=(self.pspec,) * (n_params + n_outs),
                      out_specs=(self.pspec,) * n_outs, check_rep=False),
            donate_argnums=tuple(range(n_params, n_params + n_outs)),
            keep_unused=True)

    def place(self, in_maps):
        sh = self.jax.sharding.NamedSharding(self.mesh, self.pspec)
        return [self.jax.device_put(
                    np.concatenate([np.asarray(in_maps[c][n]) for c in range(self.n_cores)], axis=0), sh)
                for n in self.in_names]

    def run(self, args):
        sh = self.jax.sharding.NamedSharding(self.mesh, self.pspec)
        zeros = [self.jax.device_put(
                    np.zeros((self.n_cores * z.shape[0], *z.shape[1:]), z.dtype), sh)
                 for z in self.zero_outs]
        outs = self.fn(*args, *zeros)
        self.jax.block_until_ready(outs)
        return outs

    def results(self, outs):
        return [{n: np.asarray(outs[i]).reshape(self.n_cores, *self.out_avals[i].shape)[c]
                 for i, n in enumerate(self.out_names)}
                for c in range(self.n_cores)]


# --------------------------------------------------------------------------
def _get(edge_index):
    key = hash(np.asarray(edge_index)[:, ::997].tobytes())
    if key not in _CACHE:
        pre = _preprocess(edge_index)
        nc = _build(pre)
        _CACHE[key] = (pre, _Runner(nc))
    return _CACHE[key]


def kernel(x, edge_index, W1, b1, W2, b2, W3, b3):
    pre, runner = _get(edge_index)
    x = np.asarray(x, np.float32)
    ids = pre["ids"]
    in_maps = []
    for c in range(NCORES):
        v = ids[c]
        xc = np.zeros((SLOTS, DIMS[0]), np.float32)
        m = v >= 0
        xc[m] = x[v[m]]
        in_maps.append({
            "xT": np.ascontiguousarray(xc.T),
            "gidx": pre["wrapped"][c],
            "dinv": pre["dinv_t"][c],
            "W1": np.asarray(W1, np.float32), "W2": np.asarray(W2, np.float32),
            "W3": np.asarray(W3, np.float32),
            "b1": np.tile(np.asarray(b1, np.float32)[None, :], (P, 1)),
            "b2": np.tile(np.asarray(b2, np.float32)[None, :], (P, 1)),
            "b3": np.tile(np.asarray(b3, np.float32)[None, :], (P, 1)),
        })
    args = runner.place(in_maps)
    outs = runner.run(args)
    res = runner.results(outs)
    full = np.zeros((N, DIMS[3]), np.float32)
    for c in range(NCORES):
        v = ids[c]
        m = v >= 0
        full[v[m]] = res[c]["out"][m]
    return full


# revision 7
# speedup vs baseline: 671.7447x; 576.6697x over previous
"""3-layer GCN (PyG GCNConv x3, relu between) on 8 Trainium2 NeuronCores.

Math: out = A*(relu(A*(relu(A*(xW1)+b1)W2+b2))W3)+b3 with A = D^-1/2(A+I)D^-1/2.
The edge norm factorizes as dinv[src]*dinv[dst], so per layer we compute
htilde = dinv * (input @ W) (dense, PE), AllGather htilde across the 8 cores,
then aggregation is an unweighted gather+segment-sum of htilde rows followed
by a dinv post-scale (+bias, relu). Nodes are degree-sorted and dealt
round-robin across cores so every core owns 12544 dst slots (98 blocks of
128 lanes) with near-uniform per-block degree; per-(block, int16-quarter)
gather slot lists are k-major so dma_gather lands edge k of lane p at
SBUF[p, k] and a single strided tensor_reduce does the segment sum. Padding
slots point at a per-slab zero row.
"""
import sys, time
sys.path.insert(0, "/opt/trn_rl_repo")
import numpy as np

N = 100_000
DIMS = [512, 128, 64, 32]
NCORES = 8
P = 128
SLOTS = 12544          # 98 blocks * 128 per core
NBLK = SLOTS // P      # 98
SLAB = SLOTS + 1       # +1 zero row per core slab
HROWS = NCORES * SLAB  # 100360
NQ = 4                 # int16-addressable quarters of the gathered table
QROWS = HROWS // NQ    # 25090
ZIDX = SLOTS           # quarter-local zero row (slab 2q, row 12544)
MAX_CALL_IDX = 3072
GDIMS = [128, 64, 64]  # gather widths (L3 padded 32->64 for 256B stride)

_CACHE = {}


# --------------------------------------------------------------------------
# host-side graph preprocessing
# --------------------------------------------------------------------------
def _preprocess(edge_index):
    src = np.asarray(edge_index[0], np.int64)
    dst = np.asarray(edge_index[1], np.int64)
    deg = np.bincount(dst, minlength=N).astype(np.int64) + 1  # + self loop
    dinv = (1.0 / np.sqrt(deg)).astype(np.float32)

    rank = np.argsort(-deg, kind="stable")
    pos = np.empty(N, np.int64); pos[rank] = np.arange(N)
    core_of = pos % NCORES
    slot_of = pos // NCORES

    S = np.concatenate([src, np.arange(N)])
    D_ = np.concatenate([dst, np.arange(N)])
    ec, eslot = core_of[D_], slot_of[D_]
    eblk, elane = eslot // P, eslot % P
    hrow = core_of[S] * SLAB + slot_of[S]
    eq, eqidx = hrow // QROWS, hrow % QROWS

    key = ((ec * NQ + eq) * NBLK + eblk) * P + elane
    order = np.argsort(key, kind="stable")
    ks = key[order]
    newgrp = np.r_[True, ks[1:] != ks[:-1]]
    first = np.flatnonzero(newgrp)
    within = np.arange(len(ks)) - first[np.cumsum(newgrp) - 1]

    cnt = np.zeros(NCORES * NQ * NBLK * P, np.int64)
    np.add.at(cnt, key, 1)
    K = cnt.reshape(NCORES, NQ, NBLK, P).max(axis=(0, 3))  # [NQ, NBLK]
    K = np.maximum(K, 1)

    packs = []  # (q, j0, j1, col_offs, nidx)
    for q in range(NQ):
        j = 0
        while j < NBLK:
            tot, j1, offs = 0, j, []
            while j1 < NBLK and (tot + K[q, j1]) * P <= MAX_CALL_IDX:
                offs.append(tot); tot += int(K[q, j1]); j1 += 1
            if j1 == j:  # single oversized block
                offs, tot, j1 = [0], int(K[q, j]), j + 1
            packs.append((q, j, j1, offs, tot * P))
            j = j1
    packs.sort(key=lambda pk: (pk[1], pk[0]))

    blkq = [[None] * NQ for _ in range(NBLK)]
    cidm = np.full((NQ, NBLK), -1, np.int64)
    offm = np.zeros((NQ, NBLK), np.int64)
    cbase = np.zeros(len(packs), np.int64)
    acc = 0
    for cid, (q, j0, j1, offs, nidx) in enumerate(packs):
        cbase[cid] = acc; acc += nidx
        for t, j in enumerate(range(j0, j1)):
            blkq[j][q] = (cid, offs[t], int(K[q, j]))
            cidm[q, j], offm[q, j] = cid, offs[t]
    total_idx = acc

    req = np.full((NCORES, total_idx), ZIDX, np.int64)
    oc, oq, ob, ol = ec[order], eq[order], eblk[order], elane[order]
    ecall = cidm[oq, ob]
    ekoff = offm[oq, ob] + within
    req[oc, cbase[ecall] + ekoff * P + ol] = eqidx[order]
    assert req.max() < 32768

    wrapped = np.empty((NCORES, P, total_idx // 16), np.int16)
    for c in range(NCORES):
        col = 0
        for cid, pk in enumerate(packs):
            L = req[c, cbase[cid]:cbase[cid] + pk[4]]
            w = L.reshape(-1, 16).T.astype(np.int16)
            wrapped[c, :, col:col + pk[4] // 16] = np.tile(w, (8, 1))
            col += pk[4] // 16

    ids = np.full((NCORES, SLOTS), -1, np.int64)
    ids[core_of, slot_of] = np.arange(N)
    dinv_t = np.zeros((NCORES, P, NBLK), np.float32)
    for c in range(NCORES):
        v = ids[c]
        dv = np.where(v >= 0, dinv[np.maximum(v, 0)], 0.0).astype(np.float32)
        dinv_t[c] = dv.reshape(NBLK, P).T
    return dict(ids=ids, packs=packs, blkq=blkq, cbase=cbase,
                wrapped=wrapped, dinv_t=dinv_t, total_idx=total_idx)


# --------------------------------------------------------------------------
# bass program
# --------------------------------------------------------------------------
def _build(pre):
    from concourse import bass, bacc, mybir, tile
    from concourse.library_config import mlp
    from concourse.masks import make_identity
    AL = mybir.AluOpType
    f32, i16 = mybir.dt.float32, mybir.dt.int16
    packs, blkq, cbase = pre["packs"], pre["blkq"], pre["cbase"]
    total_idx = pre["total_idx"]

    nc = bacc.Bacc("TRN2", target_bir_lowering=False, debug=False,
                   num_devices=NCORES)
    xT_in = nc.dram_tensor("xT", (DIMS[0], SLOTS), f32, kind="ExternalInput")
    idx_in = nc.dram_tensor("gidx", (P, total_idx // 16), i16, kind="ExternalInput")
    dinv_in = nc.dram_tensor("dinv", (P, NBLK), f32, kind="ExternalInput")
    w_in = [nc.dram_tensor(f"W{i+1}", (DIMS[i], DIMS[i + 1]), f32, kind="ExternalInput") for i in range(3)]
    b_in = [nc.dram_tensor(f"b{i+1}", (P, DIMS[i + 1]), f32, kind="ExternalInput") for i in range(3)]
    out_t = nc.dram_tensor("out", (SLOTS, DIMS[3]), f32, kind="ExternalOutput")

    with tile.TileContext(nc) as tc:
        with tc.tile_pool(name="const", bufs=1) as const, \
             tc.tile_pool(name="gath", bufs=8) as gpool, \
             tc.tile_pool(name="work", bufs=3) as work, \
             tc.tile_pool(name="lhs", bufs=3) as lhs, \
             tc.tile_pool(name="pps", bufs=2, space="PSUM") as pps, \
             tc.tile_pool(name="ppt", bufs=2, space="PSUM") as ppt, \
             tc.tile_pool(name="dram", bufs=1, space="DRAM") as dram:

            nc.gpsimd.load_library(mlp)
            idx_t = const.tile([P, total_idx // 16], i16, tag="idx")
            nc.sync.dma_start(out=idx_t[:], in_=idx_in[:, :])
            dinv_t = const.tile([P, NBLK], f32, tag="dinv")
            nc.sync.dma_start(out=dinv_t[:], in_=dinv_in[:, :])
            ident = const.tile([P, P], f32, tag="ident")
            make_identity(nc, ident[:])
            zt = const.tile([P, P], f32, tag="zero")
            nc.vector.memset(zt[:], 0.0)
            w1t = [const.tile([P, DIMS[1]], f32, tag=f"w1_{k}", name=f"w1_{k}") for k in range(4)]
            for k in range(4):
                nc.sync.dma_start(out=w1t[k][:], in_=w_in[0][k * P:(k + 1) * P, :])
            w2t = const.tile([DIMS[1], DIMS[2]], f32, tag="w2")
            nc.sync.dma_start(out=w2t[:], in_=w_in[1][:, :])
            w3t = const.tile([DIMS[2], DIMS[3]], f32, tag="w3")
            nc.sync.dma_start(out=w3t[:], in_=w_in[2][:, :])
            bt = []
            for i in range(3):
                t = const.tile([P, DIMS[i + 1]], f32, tag=f"b{i}", name=f"bt{i}")
                nc.sync.dma_start(out=t[:], in_=b_in[i][:, :])
                bt.append(t)

            slabs = [dram.tile([SLAB, GDIMS[i]], f32, tag=f"slab{i}", name=f"slab{i}") for i in range(3)]
            hbufs = [dram.tile([HROWS, GDIMS[i]], f32, tag=f"hbuf{i}", name=f"hbuf{i}", addr_space="Shared") for i in range(3)]

            def aggregate(layer):
                d = GDIMS[layer]
                nc.gpsimd.collective_compute(
                    "AllGather", AL.bypass,
                    replica_groups=[list(range(NCORES))],
                    ins=[slabs[layer].opt()], outs=[hbufs[layer].opt()])
                tiles = {}
                for cid, (q, j0, j1, offs, nidx) in enumerate(packs):
                    gt = gpool.tile([P, nidx // P, d], f32, tag="gt")
                    nc.gpsimd.dma_gather(
                        out_ap=gt[:],
                        in_ap=hbufs[layer][q * QROWS:(q + 1) * QROWS, :],
                        idxs_ap=idx_t[:, cbase[cid] // 16:(cbase[cid] + nidx) // 16],
                        num_idxs=nidx, num_idxs_reg=nidx, elem_size=d,
                        single_packet=False)
                    tiles[cid] = gt
                return tiles

            def reduce_block(tiles, j, dout):
                s = work.tile([P, dout], f32, tag="ssum")
                for q in range(NQ):
                    cid, coff, kq = blkq[j][q]
                    gt = tiles[cid]
                    view = gt[:, coff:coff + kq, :dout].rearrange("p k d -> p d k")
                    if q == 0:
                        nc.vector.tensor_reduce(out=s[:], in_=view,
                            axis=mybir.AxisListType.X, op=AL.add)
                    else:
                        tmp = work.tile([P, dout], f32, tag="rtmp")
                        nc.vector.tensor_reduce(out=tmp[:], in_=view,
                            axis=mybir.AxisListType.X, op=AL.add)
                        nc.vector.tensor_tensor(out=s[:], in0=s[:], in1=tmp[:], op=AL.add)
                return s

            def dscale(dst_ap, src_ap, j, d):
                nc.vector.tensor_tensor(
                    out=dst_ap, in0=src_ap,
                    in1=dinv_t[:, j:j + 1].to_broadcast([P, d]), op=AL.mult)

            # ---- L1 transform
            for j in range(NBLK):
                ps = pps.tile([P, DIMS[1]], f32, space="PSUM", tag="tps")
                for k in range(4):
                    lt = lhs.tile([P, P], f32, tag="xT")
                    nc.sync.dma_start(out=lt[:], in_=xT_in[k * P:(k + 1) * P, j * P:(j + 1) * P])
                    nc.tensor.matmul(out=ps[:], lhsT=lt[:], rhs=w1t[k][:],
                                     start=(k == 0), stop=(k == 3))
                ht = work.tile([P, DIMS[1]], f32, tag="hrow")
                dscale(ht[:], ps[:], j, DIMS[1])
                nc.sync.dma_start(out=slabs[0][j * P:(j + 1) * P, :], in_=ht[:])
            nc.sync.dma_start(out=slabs[0][SLOTS:SLOTS + 1, :], in_=zt[0:1, :GDIMS[0]])

            # ---- L1 aggregate + L2 transform
            tiles = aggregate(0)
            for j in range(NBLK):
                s = reduce_block(tiles, j, DIMS[1])
                dscale(s[:], s[:], j, DIMS[1])
                nc.vector.tensor_tensor(out=s[:], in0=s[:], in1=bt[0][:], op=AL.add)
                nc.vector.tensor_scalar_max(out=s[:], in0=s[:], scalar1=0.0)
                pt = ppt.tile([P, P], f32, space="PSUM", tag="trp")
                nc.tensor.transpose(out=pt[:DIMS[1], :], in_=s[:], identity=ident[:])
                sT = work.tile([DIMS[1], P], f32, tag="sT")
                nc.vector.tensor_copy(out=sT[:], in_=pt[:DIMS[1], :])
                ps = pps.tile([P, DIMS[2]], f32, space="PSUM", tag="tps2")
                nc.tensor.matmul(out=ps[:], lhsT=sT[:], rhs=w2t[:], start=True, stop=True)
                ht = work.tile([P, DIMS[2]], f32, tag="h2row")
                dscale(ht[:], ps[:], j, DIMS[2])
                nc.sync.dma_start(out=slabs[1][j * P:(j + 1) * P, :], in_=ht[:])
            nc.sync.dma_start(out=slabs[1][SLOTS:SLOTS + 1, :], in_=zt[0:1, :GDIMS[1]])

            # ---- L2 aggregate + L3 transform
            tiles = aggregate(1)
            for j in range(NBLK):
                s = reduce_block(tiles, j, DIMS[2])
                dscale(s[:], s[:], j, DIMS[2])
                nc.vector.tensor_tensor(out=s[:], in0=s[:], in1=bt[1][:], op=AL.add)
                nc.vector.tensor_scalar_max(out=s[:], in0=s[:], scalar1=0.0)
                pt = ppt.tile([P, P], f32, space="PSUM", tag="trp")
                nc.tensor.transpose(out=pt[:DIMS[2], :], in_=s[:], identity=ident[:])
                sT = work.tile([DIMS[2], P], f32, tag="s3T")
                nc.vector.tensor_copy(out=sT[:], in_=pt[:DIMS[2], :])
                ps = pps.tile([P, DIMS[3]], f32, space="PSUM", tag="tps3")
                nc.tensor.matmul(out=ps[:], lhsT=sT[:], rhs=w3t[:], start=True, stop=True)
                ht = work.tile([P, DIMS[3]], f32, tag="h3row")
                dscale(ht[:], ps[:], j, DIMS[3])
                nc.sync.dma_start(out=slabs[2][j * P:(j + 1) * P, :DIMS[3]], in_=ht[:])
                nc.sync.dma_start(out=slabs[2][j * P:(j + 1) * P, DIMS[3]:], in_=zt[:, :GDIMS[2] - DIMS[3]])
            nc.sync.dma_start(out=slabs[2][SLOTS:SLOTS + 1, :], in_=zt[0:1, :GDIMS[2]])

            # ---- L3 aggregate -> output
            tiles = aggregate(2)
            for j in range(NBLK):
                s = reduce_block(tiles, j, DIMS[3])
                dscale(s[:], s[:], j, DIMS[3])
                nc.vector.tensor_tensor(out=s[:], in0=s[:], in1=bt[2][:], op=AL.add)
                nc.sync.dma_start(out=out_t[j * P:(j + 1) * P, :], in_=s[:])
    nc.compile()
    return nc


# --------------------------------------------------------------------------
# SPMD runner (shard_map over 8 axon cores, reusable jitted executable)
# --------------------------------------------------------------------------
class _Runner:
    def __init__(self, nc, n_cores=NCORES):
        import jax
        from jax.sharding import Mesh, PartitionSpec
        from jax.experimental.shard_map import shard_map
        from concourse import bass2jax, mybir
        bass2jax.install_neuronx_cc_hook()
        self.jax = jax
        self.n_cores = n_cores
        pname = nc.partition_id_tensor.name if nc.partition_id_tensor else None
        in_names, out_names, out_avals, zero_outs = [], [], [], []
        for alloc in nc.m.functions[0].allocations:
            if not isinstance(alloc, mybir.MemoryLocationSet):
                continue
            name = alloc.memorylocations[0].name
            if alloc.kind == "ExternalInput":
                if name != pname:
                    in_names.append(name)
            elif alloc.kind == "ExternalOutput":
                out_names.append(name)
                out_avals.append(jax.core.ShapedArray(tuple(alloc.tensor_shape), mybir.dt.np(alloc.dtype)))
                zero_outs.append(np.zeros(tuple(alloc.tensor_shape), mybir.dt.np(alloc.dtype)))
        self.in_names, self.out_names = in_names, out_names
        self.out_avals, self.zero_outs = out_avals, zero_outs
        n_params, n_outs = len(in_names), len(out_names)
        all_in = in_names + out_names + ([pname] if pname else [])

        def _body(*args):
            operands = list(args)
            if pname:
                operands.append(bass2jax.partition_id_tensor())
            outs = bass2jax._bass_exec_p.bind(
                *operands, out_avals=tuple(out_avals), in_names=tuple(all_in),
                out_names=tuple(out_names), lowering_input_output_aliases=(),
                sim_require_finite=True, sim_require_nnan=True, nc=nc)
            return tuple(outs)

        devices = jax.devices()[:n_cores]
        self.mesh = Mesh(np.asarray(devices), ("core",))
        self.pspec = PartitionSpec("core")
        self.fn = jax.jit(
            shard_map(_body, mesh=self.mesh,
                      in_specs=(self.pspec,) * (n_params + n_outs),
                      out_specs=(self.pspec,) * n_outs, check_rep=False),
            donate_argnums=tuple(range(n_params, n_params + n_outs)),
            keep_unused=True)

    def place(self, in_maps):
        sh = self.jax.sharding.NamedSharding(self.mesh, self.pspec)
        return [self.jax.device_put(
                    np.concatenate([np.asarray(in_maps[c][n]) for c in range(self.n_cores)], axis=0), sh)
                for n in self.in_names]

    def make_zeros(self):
        sh = self.jax.sharding.NamedSharding(self.mesh, self.pspec)
        zeros = [self.jax.device_put(
                    np.zeros((self.n_cores * z.shape[0], *z.shape[1:]), z.dtype), sh)
                 for z in self.zero_outs]
        self.jax.block_until_ready(zeros)
        return zeros

    def run(self, args, zeros=None):
        if zeros is None:
            zeros = self.make_zeros()
        outs = self.fn(*args, *zeros)
        self.jax.block_until_ready(outs)
        return outs

    def results(self, outs):
        return [{n: np.asarray(outs[i]).reshape(self.n_cores, *self.out_avals[i].shape)[c]
                 for i, n in enumerate(self.out_names)}
                for c in range(self.n_cores)]


# --------------------------------------------------------------------------
def _get(edge_index):
    key = hash(np.asarray(edge_index)[:, ::997].tobytes())
    if key not in _CACHE:
        pre = _preprocess(edge_index)
        nc = _build(pre)
        _CACHE[key] = (pre, _Runner(nc))
    return _CACHE[key]


def kernel(x, edge_index, W1, b1, W2, b2, W3, b3):
    pre, runner = _get(edge_index)
    x = np.asarray(x, np.float32)
    ids = pre["ids"]
    in_maps = []
    for c in range(NCORES):
        v = ids[c]
        xc = np.zeros((SLOTS, DIMS[0]), np.float32)
        m = v >= 0
        xc[m] = x[v[m]]
        in_maps.append({
            "xT": np.ascontiguousarray(xc.T),
            "gidx": pre["wrapped"][c],
            "dinv": pre["dinv_t"][c],
            "W1": np.asarray(W1, np.float32), "W2": np.asarray(W2, np.float32),
            "W3": np.asarray(W3, np.float32),
            "b1": np.tile(np.asarray(b1, np.float32)[None, :], (P, 1)),
            "b2": np.tile(np.asarray(b2, np.float32)[None, :], (P, 1)),
            "b3": np.tile(np.asarray(b3, np.float32)[None, :], (P, 1)),
        })
    args = runner.place(in_maps)
    outs = runner.run(args)
    res = runner.results(outs)
    full = np.zeros((N, DIMS[3]), np.float32)
    for c in range(NCORES):
        v = ids[c]
        m = v >= 0
        full[v[m]] = res[c]["out"][m]
    return full
